# revision 1
# baseline (speedup 1.0000x reference)
"""DeepSeekV3 latent attention (MLA) on 8 TRN2 NeuronCores via Bass/Tile.

Sharding: 8 cores = 2 batches x 4 head-groups (4 heads each).
Each core: full projections for its batch (c_kv latent replicated within
batch group), attention for its 4 heads, and a W_O row-slice partial
output; host sums the 4 partials per batch.

Heavy matmuls run as float32r (TF32-like, 1 cyc/row); the softmax score
operands (qcT/qrot/krot/k_abs) are bf16 to fit SBUF - softmax output is
insensitive to small logit noise. Layouts are feature-major so no PE
transposes are needed:
  cT[l,t], qcT[d,s], k_absT[d,t] = wuk^T x cT, v_abs[t,d4] = cT^T x wuv,
  scoresT[t,s] = k_absT^T x qcT + k_rot^T x q_rot, p = exp(scoresT),
  ctxT[d,s] = (v_abs^T x p) * (1/sums), outT[din,s] = wO^T x ctx_flat.
Softmax sums come from an all-ones stationary matmul (broadcast rows);
causal masking = tile skipping + one 128x128 binary mask on diagonal
sub-tiles. RoPE uses host cos/sin tables + a DMA partition half-swap.
"""

import math
import ml_dtypes
import numpy as np

import concourse.bacc as bacc
import concourse.tile as tile
from concourse import mybir

FP32 = mybir.dt.float32
FP32R = mybir.dt.float32r
BF16 = mybir.dt.bfloat16

B, S, DIN = 2, 2048, 2048
DOUT, H, RD, L = 2048, 16, 64, 512
HD = DOUT // H  # 128
ROPE_BASE = 10000.0
EPS = 1e-6
HPG = 4          # heads per group (per core)
NCORES = 8
SB = 4           # s blocks of 512
NT = 16          # token tiles of 128
ND = 16          # din chunks of 128
NF = 12          # stage-1 feature tiles: 4 c + 4 qc + 2 qrope + 2 krope
FW = NF * 128    # 1536 stage-1 projection output features

_cache = {}


def _build():
    nc = bacc.Bacc("TRN2", target_bir_lowering=False, debug=False)

    xT = nc.dram_tensor("xT", [DIN, S], FP32R, kind="ExternalInput").ap()
    wP = nc.dram_tensor("wP", [DIN, FW], FP32R, kind="ExternalInput").ap()
    wUK = nc.dram_tensor("wUK", [L, 512], FP32R, kind="ExternalInput").ap()
    wUV = nc.dram_tensor("wUV", [L, 512], FP32R, kind="ExternalInput").ap()
    wO = nc.dram_tensor("wO", [512, DIN], FP32R, kind="ExternalInput").ap()
    ctab = nc.dram_tensor("ctab", [128, S], BF16, kind="ExternalInput").ap()
    stab = nc.dram_tensor("stab", [128, S], BF16, kind="ExternalInput").ap()
    onesd = nc.dram_tensor("onesd", [128, 128], FP32R, kind="ExternalInput").ap()
    onesb = nc.dram_tensor("onesb", [128, 128], BF16, kind="ExternalInput").ap()
    maskd = nc.dram_tensor("maskd", [128, 128], FP32, kind="ExternalInput").ap()
    epsd = nc.dram_tensor("epsd", [128, 1], FP32, kind="ExternalInput").ap()
    outT = nc.dram_tensor("outT", [DIN, S], FP32, kind="ExternalOutput").ap()

    # [p, c, s] views for single-instruction strided loads
    xT_v = xT.rearrange("(c p) s -> p c s", p=128)
    wP_v = wP.rearrange("(c p) f -> p c f", p=128)

    with tile.TileContext(nc) as tc:
        cst = tc.alloc_tile_pool(name="cst", bufs=1)
        psg = tc.alloc_tile_pool(name="psg", bufs=8, space="PSUM")
        ones_t = cst.tile([128, 128], FP32R, tag="ones", name="ones_t")
        nc.sync.dma_start(ones_t[:], onesd[:])
        mask_t = cst.tile([128, 128], FP32, tag="mask", name="mask_t")
        nc.sync.dma_start(mask_t[:], maskd[:])
        eps_t = cst.tile([128, 1], FP32, tag="eps", name="eps_t")
        nc.sync.dma_start(eps_t[:], epsd[:])
        onesb_t = cst.tile([128, 128], BF16, tag="onesb", name="onesb_t")
        nc.sync.dma_start(onesb_t[:], onesb[:])

        proj = tc.alloc_tile_pool(name="proj", bufs=1)
        qcT = [proj.tile([128, S], BF16, tag=f"qcT{j}", name=f"qcT{j}")
               for j in range(HPG)]
        qrot = [proj.tile([128, S], BF16, tag=f"qrot{p}", name=f"qrot{p}")
                for p in range(2)]
        krot = [proj.tile([128, S], BF16, tag=f"krot{p}", name=f"krot{p}")
                for p in range(2)]

        cpool = tc.alloc_tile_pool(name="cpool", bufs=1)
        cT = [cpool.tile([128, S], FP32R, tag=f"cT{lc}", name=f"cT{lc}")
              for lc in range(4)]

        # ---------------- stage 1: projections (feature-major) --------
        # half 0: features 0:768  = c(4 tiles) + qc0 + qc1
        # half 1: features 768:1536 = qc2 + qc3 + qrope(2) + krope(2)
        whp = tc.alloc_tile_pool(name="wh", bufs=2)
        xtp = tc.alloc_tile_pool(name="xt", bufs=2)
        s1t = tc.alloc_tile_pool(name="s1t", bufs=1)
        tabs = tc.alloc_tile_pool(name="tabs", bufs=2)

        wabs = tc.alloc_tile_pool(name="wabs", bufs=1, side="right")
        wuk = [wabs.tile([128, 512], FP32R, tag=f"wuk{lc}", name=f"wuk{lc}")
               for lc in range(4)]
        wuv = [wabs.tile([128, 512], FP32R, tag=f"wuv{lc}", name=f"wuv{lc}")
               for lc in range(4)]

        craws = {}
        whh = {}
        for half in range(2):
            if half == 0:
                # warm the xt pipeline before the big weight DMAs
                xt0 = xtp.tile([128, 8 * 512], FP32R, tag="xt", name="xt")
                xv0 = xt0[:].rearrange("p (c s) -> p c s", s=512)
                for lo, hi in ((0, 1), (1, 4), (4, 8)):
                    nc.gpsimd.dma_start(
                        xv0[:, lo:hi, :],
                        xT_v[:, lo:hi, 0:512],
                    )
                # absorption weights: needed only at stage 2, queue behind
                # the xt warm-up so the ACT DMA queue stays free at start
                for lc in range(4):
                    nc.gpsimd.dma_start(wuk[lc][:],
                                        wUK[lc * 128:(lc + 1) * 128, :])
                    nc.gpsimd.dma_start(wuv[lc][:],
                                        wUV[lc * 128:(lc + 1) * 128, :])
            for dh in range(2):
                w = whp.tile([128, 8 * 768], FP32R, tag="wh", name="wh")
                wv = w[:].rearrange("p (c f) -> p c f", f=768)
                bounds = [0, 1, 2, 4, 6, 8] if dh == 0 else [0, 2, 4, 6, 8]
                for bi in range(len(bounds) - 1):
                    lo, hi = bounds[bi], bounds[bi + 1]
                    eng = nc.scalar if bi == 0 else nc.sync
                    eng.dma_start(
                        wv[:, lo:hi, :],
                        wP_v[:, dh * 8 + lo:dh * 8 + hi,
                             half * 768:(half + 1) * 768],
                    )
                whh[(half, dh)] = w
            for sb in range(SB):
                sc = slice(sb * 512, (sb + 1) * 512)
                if half == 1:
                    rrall = s1t.tile([128, 2048], BF16, tag="rrall",
                                     name="rrall")
                for cblk in range(2):
                    if half == 0 and sb == 0 and cblk == 0:
                        xt = xt0
                    else:
                        xt = xtp.tile([128, 8 * 512], FP32R, tag="xt",
                                      name="xt")
                        xv = xt[:].rearrange("p (c s) -> p c s", s=512)
                        for qq in range(2):
                            # alternate queues so activations are not
                            # serialized behind the weight bulk stream
                            eng = nc.scalar if (sb + cblk + qq) % 2 else nc.sync
                            eng.dma_start(
                                xv[:, qq * 4:(qq + 1) * 4, :],
                                xT_v[:, cblk * 8 + qq * 4:
                                     cblk * 8 + (qq + 1) * 4, sc],
                            )
                    for fi in range(6):
                        ft = half * 6 + fi
                        if cblk == 0:
                            pt = psg.tile([128, 512], FP32,
                                          tag="acc", name=f"acc{ft}")
                            craws[ft] = pt
                        else:
                            pt = craws[ft]
                        for c8 in range(8):
                            c = cblk * 8 + c8
                            nc.tensor.matmul(
                                pt[:],
                                whh[(half, cblk)][:, c8 * 768 + fi * 128:
                                                  c8 * 768 + (fi + 1) * 128],
                                xt[:, c8 * 512:(c8 + 1) * 512],
                                start=(c == 0),
                                stop=(c == ND - 1),
                            )
                        if cblk == 1:
                            # inline evacuation frees the PSUM slot early
                            if ft < 4:
                                craw_sb = s1t.tile([128, 512], FP32,
                                                   tag=f"craw{ft}",
                                                   name=f"craw{ft}")
                                craws[f"s{ft}"] = craw_sb
                                nc.scalar.copy(craw_sb[:], pt[:])
                            elif ft < 8:
                                nc.scalar.copy(qcT[ft - 4][:, sc], pt[:])
                            else:
                                nc.scalar.copy(
                                    rrall[:, (ft - 8) * 512:(ft - 7) * 512],
                                    pt[:],
                                )
                if half == 0:
                    craw = [craws[f"s{lc}"] for lc in range(4)]
                    # rmsnorm for this s block
                    sq = [
                        s1t.tile([128, 512], BF16, tag=f"sq{lc}",
                                 name=f"sq{lc}")
                        for lc in range(4)
                    ]
                    for lc in range(4):
                        nc.vector.tensor_tensor(
                            out=sq[lc][:], in0=craw[lc][:], in1=craw[lc][:],
                            op=mybir.AluOpType.mult,
                        )
                    msum = psg.tile([128, 512], FP32, tag="acc", name="msum")
                    for lc in range(4):
                        nc.tensor.matmul(
                            msum[:], onesb_t[:], sq[lc][:],
                            start=(lc == 0), stop=(lc == 3),
                        )
                    rs = s1t.tile([128, 512], FP32, tag="rs", name="rs")
                    nc.scalar.activation(
                        rs[:], msum[:], mybir.ActivationFunctionType.Sqrt,
                        bias=eps_t[:], scale=1.0 / L,
                    )
                    rsr = s1t.tile([128, 512], FP32, tag="rsr", name="rsr")
                    nc.vector.reciprocal(rsr[:], rs[:])
                    for lc in range(4):
                        nc.vector.tensor_tensor(
                            out=cT[lc][:, sc], in0=craw[lc][:], in1=rsr[:],
                            op=mybir.AluOpType.mult,
                        )
                else:
                    ctab_t = tabs.tile([128, 512], BF16, tag="ctab",
                                       name="ctab_t")
                    nc.sync.dma_start(ctab_t[:], ctab[:, sc])
                    stab_t = tabs.tile([128, 512], BF16, tag="stab",
                                       name="stab_t")
                    nc.sync.dma_start(stab_t[:], stab[:, sc])
                    rswall = s1t.tile([128, 2048], BF16, tag="rswall",
                                      name="rswall")
                    for q in range(4):
                        srcb = (q ^ 1) * 32
                        nc.gpsimd.dma_start(
                            rswall[q * 32:(q + 1) * 32, :],
                            rrall[srcb:srcb + 32, :],
                        )
                    for fi in range(8, 12):
                        fc = slice((fi - 8) * 512, (fi - 7) * 512)
                        t1 = s1t.tile([128, 512], BF16, tag="t1", name="t1")
                        nc.vector.tensor_tensor(
                            out=t1[:], in0=rrall[:, fc], in1=ctab_t[:],
                            op=mybir.AluOpType.mult,
                        )
                        t2 = s1t.tile([128, 512], BF16, tag="t2", name="t2")
                        nc.vector.tensor_tensor(
                            out=t2[:], in0=rswall[:, fc], in1=stab_t[:],
                            op=mybir.AluOpType.mult,
                        )
                        dst = (qrot if fi < 10 else krot)[fi % 2]
                        nc.vector.tensor_tensor(
                            out=dst[:, sc], in0=t1[:], in1=t2[:],
                            op=mybir.AluOpType.add,
                        )

        tabs.release()
        s1t.release()
        xtp.release()
        whp.release()

        # ---------------- stage 2: K/V absorption ---------------------
        absp = tc.alloc_tile_pool(name="absp", bufs=1, side="right")
        kabs = [absp.tile([128, S], BF16, tag=f"kabs{j}", name=f"kabs{j}")
                for j in range(HPG)]
        vabs = [absp.tile([128, 512], FP32R, tag=f"vabs{t}", name=f"vabs{t}")
                for t in range(NT)]

        for tb in range(SB):
            tcs = slice(tb * 512, (tb + 1) * 512)
            for g in range(HPG):
                # alternate ACT-evac'd kabs and DVE-evac'd vabs groups so
                # PSUM slot releases stream on both engines
                pt = psg.tile([128, 512], FP32, tag="acc", name="pabs")
                for lc in range(4):
                    nc.tensor.matmul(
                        pt[:],
                        wuk[lc][:, g * 128:(g + 1) * 128],
                        cT[lc][:, tcs],
                        start=(lc == 0), stop=(lc == 3),
                    )
                nc.scalar.copy(kabs[g][:, tcs], pt[:])
                tt = tb * 4 + g
                pt = psg.tile([128, 512], FP32, tag="acc", name="pabs")
                for lc in range(4):
                    nc.tensor.matmul(
                        pt[:],
                        cT[lc][:, tt * 128:(tt + 1) * 128],
                        wuv[lc][:],
                        start=(lc == 0), stop=(lc == 3),
                    )
                nc.vector.tensor_copy(vabs[tt][:], pt[:])

        cpool.release()

        # ---------------- stage 3: attention, head pairs --------------
        ctxp = tc.alloc_tile_pool(name="ctxp", bufs=1)
        ctx = [ctxp.tile([128, S], FP32R, tag=f"ctx{j}", name=f"ctx{j}")
               for j in range(HPG)]
        # prefetch the stage-4 weights during stage 3 (right side pool)
        wop = tc.alloc_tile_pool(name="wop", bufs=1)
        wo = [wop.tile([128, DIN], FP32R, tag=f"wo{j}", name=f"wo{j}")
              for j in range(HPG)]
        for j in range(HPG):
            nc.sync.dma_start(wo[j][:], wO[j * 128:(j + 1) * 128, :])

        s3t = tc.alloc_tile_pool(name="s3t", bufs=8)
        s3r = tc.alloc_tile_pool(name="s3r", bufs=2)
        for jp in range(2):
            ja, jb = 2 * jp, 2 * jp + 1
            rp = jp
            for sb in range(SB):
                sc = slice(sb * 512, (sb + 1) * 512)
                cxa = psg.tile([128, 512], FP32, tag="acc", name="cxa")
                cxb = psg.tile([128, 512], FP32, tag="acc", name="cxb")
                sma = psg.tile([128, 512], FP32, tag="acc", name="sma")
                smb = psg.tile([128, 512], FP32, tag="acc", name="smb")
                nti = 4 * sb + 4
                LAG = 3
                pfs = {}
                # software pipeline: PV/sums lag scores by LAG tiles
                for ti in range(nti + LAG):
                    if ti < nti:
                        j0 = ti - 4 * sb
                        c0 = max(j0, 0) * 128
                        sca = psg.tile([128, 512], FP32, tag="acc",
                                        name="sca")
                        scb = psg.tile([128, 512], FP32, tag="acc",
                                        name="scb")
                        for (jj, scp, rrow) in ((ja, sca, 0), (jb, scb, 64)):
                            nc.tensor.matmul(
                                scp[:, c0:512],
                                kabs[jj][:, ti * 128:(ti + 1) * 128],
                                qcT[jj][:, sb * 512 + c0:(sb + 1) * 512],
                                start=True, stop=False,
                            )
                        # adjacent K=64 matmuls in disjoint row groups
                        for (jj, scp, rrow) in ((ja, sca, 0), (jb, scb, 64)):
                            nc.tensor.matmul(
                                scp[:, c0:512],
                                krot[rp][rrow:rrow + 64,
                                         ti * 128:(ti + 1) * 128],
                                qrot[rp][rrow:rrow + 64,
                                         sb * 512 + c0:(sb + 1) * 512],
                                start=False, stop=True,
                            )
                        pfpair = []
                        for (jj, scp) in ((ja, sca), (jb, scb)):
                            p = s3t.tile([128, 512], FP32, tag="p", name="p")
                            nc.scalar.activation(
                                p[:, c0:512], scp[:, c0:512],
                                mybir.ActivationFunctionType.Exp,
                            )
                            pf = s3t.tile([128, 512], FP32R, tag="pf",
                                          name="pf")
                            if j0 >= 0:
                                nc.vector.tensor_tensor(
                                    out=pf[:, c0:c0 + 128],
                                    in0=p[:, c0:c0 + 128], in1=mask_t[:],
                                    op=mybir.AluOpType.mult,
                                )
                                if c0 + 128 < 512:
                                    nc.vector.tensor_copy(
                                        pf[:, c0 + 128:512],
                                        p[:, c0 + 128:512],
                                    )
                            else:
                                nc.vector.tensor_copy(pf[:], p[:])
                            pfpair.append(pf)
                        pfs[ti] = (pfpair, c0)
                    if ti >= LAG:
                        tv = ti - LAG
                        (pfa, pfb), c0 = pfs.pop(tv)
                        for (jj, pf, cxp) in (
                            (ja, pfa, cxa), (jb, pfb, cxb)
                        ):
                            nc.tensor.matmul(
                                cxp[:, c0:512],
                                vabs[tv][:, jj * 128:(jj + 1) * 128],
                                pf[:, c0:512],
                                start=(tv == 0), stop=(tv == nti - 1),
                            )
                        for (pf, smp) in ((pfa, sma), (pfb, smb)):
                            nc.tensor.matmul(
                                smp[:, c0:512],
                                ones_t[:],
                                pf[:, c0:512],
                                start=(tv == 0), stop=(tv == nti - 1),
                            )
                for (jj, cxp, smp) in ((ja, cxa, sma), (jb, cxb, smb)):
                    rec = s3r.tile([128, 512], FP32, tag="rec", name="rec")
                    nc.vector.reciprocal(rec[:], smp[:])
                    nc.vector.tensor_tensor(
                        out=ctx[jj][:, sc], in0=cxp[:], in1=rec[:],
                        op=mybir.AluOpType.mult,
                    )
        s3r.release()
        s3t.release()
        absp.release()

        # ---------------- stage 4: output projection ------------------
        s4t = tc.alloc_tile_pool(name="s4t", bufs=4)
        # sb-pair-outer: output projection for s-blocks (0,1) starts as
        # soon as stage-3 finishes those columns (overlaps head pair 1)
        for sbp in range(2):
            for dt in range(ND):
                dc = slice(dt * 128, (dt + 1) * 128)
                oc = slice(sbp * 1024, (sbp + 1) * 1024)
                orow = s4t.tile([128, 1024], FP32, tag="orow", name="orow")
                for sh in range(2):
                    sb = 2 * sbp + sh
                    sc = slice(sb * 512, (sb + 1) * 512)
                    pt = psg.tile([128, 512], FP32, tag="acc", name="po")
                    for j in range(HPG):
                        nc.tensor.matmul(
                            pt[:], wo[j][:, dc], ctx[j][:, sc],
                            start=(j == 0), stop=(j == HPG - 1),
                        )
                    nc.scalar.copy(orow[:, sh * 512:(sh + 1) * 512], pt[:])
                nc.gpsimd.dma_start(outT[dc, oc], orow[:])
        s4t.release()
        wop.release()
        ctxp.release()
        proj.release()
        wabs.release()
        psg.release()
        cst.release()

    nc.compile()
    return nc


def _host_inputs(core, x, W_DKV, W_KRope, W_Q, W_UK, W_UV, W_O, kv_norm_w, offset):
    b, g = core // HPG, core % HPG
    scale = 1.0 / math.sqrt(HD + RD)
    qc_rows = W_Q[512 * g:512 * (g + 1)] * scale
    qr_rows = W_Q[DOUT + 256 * g:DOUT + 256 * (g + 1)] * scale
    kr_rows = W_KRope[256 * g:256 * (g + 1)]
    wP = np.concatenate([W_DKV, qc_rows, qr_rows, kr_rows], axis=0).T
    wUK = (W_UK[512 * g:512 * (g + 1)] * kv_norm_w[None, :]).T
    wUV = (W_UV[512 * g:512 * (g + 1)] * kv_norm_w[None, :]).T
    wO = W_O[:, 512 * g:512 * (g + 1)].T

    pos = (float(offset) + np.arange(S, dtype=np.float64))
    inv = 1.0 / ROPE_BASE ** (np.arange(0, RD, 2, dtype=np.float64) / RD)
    ang = pos[:, None] * inv[None, :]               # [S, 32]
    cos64 = np.concatenate([np.cos(ang), np.cos(ang)], axis=1).T  # [64,S]
    st64 = np.concatenate([-np.sin(ang), np.sin(ang)], axis=1).T
    ctab = np.concatenate([cos64, cos64], axis=0).astype(ml_dtypes.bfloat16)
    stab = np.concatenate([st64, st64], axis=0).astype(ml_dtypes.bfloat16)

    mask = np.triu(np.ones((128, 128), dtype=np.float32))

    return {
        "xT": np.ascontiguousarray(x[b].T, dtype=np.float32),
        "wP": np.ascontiguousarray(wP, dtype=np.float32),
        "wUK": np.ascontiguousarray(wUK, dtype=np.float32),
        "wUV": np.ascontiguousarray(wUV, dtype=np.float32),
        "wO": np.ascontiguousarray(wO, dtype=np.float32),
        "ctab": ctab,
        "stab": stab,
        "onesd": np.ones((128, 128), dtype=np.float32),
        "onesb": np.ones((128, 128), dtype=ml_dtypes.bfloat16),
        "maskd": np.ascontiguousarray(mask),
        "epsd": np.full((128, 1), EPS, dtype=np.float32),
    }


def _reference_numpy(x, W_DKV, W_KRope, W_Q, W_UK, W_UV, W_O, kv_norm_w, offset):
    """Pure-numpy fallback (only used if offset != 0)."""
    x = x.astype(np.float64)
    b, s, _ = x.shape
    ckv = x @ W_DKV.T.astype(np.float64)
    ms = np.mean(ckv * ckv, axis=-1, keepdims=True)
    ckv = ckv / np.sqrt(ms + EPS) * kv_norm_w
    kr = (x @ W_KRope.T.astype(np.float64)).reshape(b, s, H, RD).transpose(0, 2, 1, 3)
    qt = x @ W_Q.T.astype(np.float64)
    qc = qt[..., :DOUT].reshape(b, s, H, HD).transpose(0, 2, 1, 3)
    qr = qt[..., DOUT:].reshape(b, s, H, RD).transpose(0, 2, 1, 3)

    def rope(t):
        pos = float(offset) + np.arange(s)
        inv = 1.0 / ROPE_BASE ** (np.arange(0, RD, 2) / RD)
        ang = pos[:, None] * inv[None, :]
        cos = np.concatenate([np.cos(ang)] * 2, -1)
        sin = np.concatenate([np.sin(ang)] * 2, -1)
        t1, t2 = t[..., :RD // 2], t[..., RD // 2:]
        rot = np.concatenate([-t2, t1], -1)
        return t * cos + rot * sin

    qr, kr = rope(qr), rope(kr)
    wuk = W_UK.reshape(H, HD, L).astype(np.float64)
    qa = np.einsum("bhsd,hdl->bhsl", qc, wuk)
    sc = (np.einsum("bhsl,btl->bhst", qa, ckv)
          + np.einsum("bhsr,bhtr->bhst", qr, kr)) / math.sqrt(HD + RD)
    causal = np.arange(s)[None, :] > (np.arange(s)[:, None] + int(offset))
    sc = np.where(causal[None, None], -np.inf, sc)
    sc = sc - sc.max(-1, keepdims=True)
    e = np.exp(sc)
    attn = e / e.sum(-1, keepdims=True)
    cl = np.einsum("bhst,btl->bhsl", attn, ckv)
    wuv = W_UV.reshape(H, HD, L).astype(np.float64)
    ctx = np.einsum("bhsl,hdl->bhsd", cl, wuv)
    ctx = ctx.transpose(0, 2, 1, 3).reshape(b, s, DOUT)
    return (ctx @ W_O.T.astype(np.float64)).astype(np.float32)


def kernel(**inputs) -> np.ndarray:
    args = {k: np.asarray(v) if not np.isscalar(v) else v for k, v in inputs.items()}
    x = np.asarray(args["x"], dtype=np.float32)
    offset = int(np.asarray(args["offset"]))
    names = ["x", "W_DKV", "W_KRope", "W_Q", "W_UK", "W_UV", "W_O", "kv_norm_w"]
    arrs = {n: np.asarray(args[n], dtype=np.float32) for n in names}

    if offset != 0:
        return _reference_numpy(**arrs, offset=offset)

    if "nc" not in _cache:
        _cache["nc"] = _build()
    nc = _cache["nc"]

    from concourse.bass_utils import run_bass_kernel_spmd

    in_maps = [
        _host_inputs(core, offset=offset, **arrs) for core in range(NCORES)
    ]
    res = run_bass_kernel_spmd(nc, in_maps, list(range(NCORES)))
    _cache["last_result"] = res

    out = np.zeros((B, S, DIN), dtype=np.float32)
    for core in range(NCORES):
        b = core // HPG
        out[b] += res.results[core]["outT"].T
    return out



# revision 49
# speedup vs baseline: 1.2735x; 1.2735x over previous
"""DeepSeekV3 latent attention (MLA) on 8 TRN2 NeuronCores via Bass/Tile.

Sharding: 8 cores = 2 batches x 4 head-groups (4 heads each). Each core
runs the full pipeline for its batch/heads and emits a W_O row-slice
partial output; the host sums the 4 partials per batch (bf16 outT).

Numerics: the value path (c_kv latent used as V, W_UV absorption, ctx,
W_O) is bf16 - its error lands directly on the output.  The query/key
path (q/k/rope projections, W_UK absorption, attention logits) is
fp8e4m3 run in DoubleRow perf mode (K=256 contractions at 0.5 cyc/row):
logit noise averages away under softmax over many tokens.  Queries in
the FIRST s-block attend few tokens (no averaging), so q-block 0 gets a
dedicated bf16 attention path (bf16 q/k projections + bf16 scores);
blocks 1-3 use fp8 packs that fuse content (128) + rope (64) into ONE
DoubleRow matmul per token tile.  DoubleRow operand slices must be
contiguous ([p, 2, N]) - strided plane slices lock up the PE - so the
q pack is blocked per s-block and the k pack per token tile.

Layout is feature-major throughout ([feat, s] / [latent, t]); rope
uses host cos/sin tables plus an SBUF-to-SBUF 32-row half-swap DMA.
Per s-block the kernel fuses: projections -> rmsnorm -> K/V absorption
-> causal attention for this q-block -> W_O partial -> DMA out.
"""

import math
import ml_dtypes
import numpy as np

import concourse.bacc as bacc
import concourse.tile as tile
from concourse import mybir

FP32 = mybir.dt.float32
BF16 = mybir.dt.bfloat16
FP8 = mybir.dt.float8e4
DR = mybir.MatmulPerfMode.DoubleRow

B, S, DIN = 2, 2048, 2048
DOUT, H, RD, L = 2048, 16, 64, 512
HD = DOUT // H  # 128
ROPE_BASE = 10000.0
EPS = 1e-6
HPG = 4          # heads per group (per core)
NCORES = 8
SB = 4           # s blocks of 512
NT = 16          # token tiles of 128
SCALE = 1.0 / math.sqrt(HD + RD)

_cache = {}


def _build():
    nc = bacc.Bacc("TRN2", target_bir_lowering=False, debug=False)

    xb = nc.dram_tensor("xb", [DIN, S], BF16, kind="ExternalInput").ap()
    x8 = nc.dram_tensor("x8", [128, 8 * 2 * S], FP8, kind="ExternalInput").ap()
    wc = nc.dram_tensor("wc", [DIN, 512], BF16, kind="ExternalInput").ap()
    w8 = nc.dram_tensor("w8", [128, 8 * 2048], FP8, kind="ExternalInput").ap()
    wqkb = nc.dram_tensor("wqkb", [DIN, 1024], BF16,
                          kind="ExternalInput").ap()
    wuk8 = nc.dram_tensor("wuk8", [128, 2 * 4 * 2 * 128], FP8,
                          kind="ExternalInput").ap()
    wukb = nc.dram_tensor("wukb", [L, 512], BF16, kind="ExternalInput").ap()
    wuv = nc.dram_tensor("wuv", [L, 512], BF16, kind="ExternalInput").ap()
    wo = nc.dram_tensor("wo", [512, DIN], BF16, kind="ExternalInput").ap()
    ctab = nc.dram_tensor("ctab", [128, S], BF16, kind="ExternalInput").ap()
    stab = nc.dram_tensor("stab", [128, S], BF16, kind="ExternalInput").ap()
    onesd = nc.dram_tensor("onesd", [128, 128], BF16, kind="ExternalInput").ap()
    maskd = nc.dram_tensor("maskd", [128, 128], BF16, kind="ExternalInput").ap()
    epsd = nc.dram_tensor("epsd", [128, 1], FP32, kind="ExternalInput").ap()
    zed = nc.dram_tensor("zed", [128, S], FP8, kind="ExternalInput").ap()
    outT = nc.dram_tensor("outT", [DIN, S], BF16, kind="ExternalOutput").ap()

    xb_v = xb.rearrange("(c p) s -> p c s", p=128)          # [128,16,S]
    x8_v = x8.rearrange("p (cp i s) -> p cp i s", cp=8, i=2)
    wc_v = wc.rearrange("(c p) f -> p c f", p=128)          # [128,16,512]
    # feature-tile-major so per-feature DMAs are contiguous
    w8_v = w8.rearrange("p (ft cp i f) -> p ft cp i f", ft=8, cp=8, i=2)
    wqkb_v = wqkb.rearrange("(c p) f -> p c f", p=128)      # [128,16,1024]
    wukb_v = wukb.rearrange("(lc p) d -> p lc d", p=128)
    wuv_v = wuv.rearrange("(lc p) d -> p lc d", p=128)      # [128,4,512]

    with tile.TileContext(nc) as tc:
        cst = tc.alloc_tile_pool(name="cst", bufs=1)
        ones_t = cst.tile([128, 128], BF16, tag="ones", name="ones_t")
        mask_t = cst.tile([128, 128], BF16, tag="mask", name="mask_t")
        eps_t = cst.tile([128, 1], FP32, tag="eps", name="eps_t")

        wgt = tc.alloc_tile_pool(name="wgt", bufs=1, side="right")
        wc_t = wgt.tile([128, 16 * 512], BF16, tag="wc", name="wc_t")
        w8_t = wgt.tile([128, 8 * 2048], FP8, tag="w8", name="w8_t")
        wuk8_t = wgt.tile([128, 2048], FP8, tag="wuk8", name="wuk8_t")
        wukb_t = wgt.tile([128, 4 * 512], BF16, tag="wukb", name="wukb_t")
        wuv_t = wgt.tile([128, 4 * 512], BF16, tag="wuv", name="wuv_t")
        wo_t = [wgt.tile([128, DIN], BF16, tag=f"wo{j}", name=f"wo{j}")
                for j in range(HPG)]
        wcv = wc_t[:].rearrange("p (c f) -> p c f", f=512)
        w8v = w8_t[:].rearrange("p (ft cp i f) -> p ft cp i f",
                                ft=8, cp=8, i=2)
        wukv = wuk8_t[:].rearrange("p (q j i d) -> p q j i d",
                                   q=2, j=4, i=2)
        wukbv = wukb_t[:].rearrange("p (lc d) -> p lc d", d=512)
        wuvv = wuv_t[:].rearrange("p (lc d) -> p lc d", d=512)

        packs = tc.alloc_tile_pool(name="packs", bufs=1)
        # qp: [p, qb(3: s-blocks 1..3), i, 512]; kp: [p, tt, i, 128].
        # DoubleRow slices [:, blk, :, :] are contiguous.
        qp = [packs.tile([128, 3 * 2 * 512], FP8, tag=f"qp{j}",
                         name=f"qp{j}") for j in range(HPG)]
        kp = [packs.tile([128, NT * 2 * 128], FP8, tag=f"kp{j}",
                         name=f"kp{j}") for j in range(HPG)]
        qpv = [t[:].rearrange("p (b i s) -> p b i s", b=3, i=2)
               for t in qp]
        kpv = [t[:].rearrange("p (b i s) -> p b i s", b=NT, i=2)
               for t in kp]

        vp = tc.alloc_tile_pool(name="vp", bufs=1)
        vabs = [vp.tile([128, 512], BF16, tag=f"vabs{t}", name=f"vabs{t}")
                for t in range(NT)]

        # ---- initial DMAs (all HWDGE via sync; Pool stays free) -----
        xs = tc.alloc_tile_pool(name="xs", bufs=5)
        x8s = tc.alloc_tile_pool(name="x8s", bufs=3)
        tabs = tc.alloc_tile_pool(name="tabs", bufs=1)
        wqs = tc.alloc_tile_pool(name="wqs", bufs=3)

        def fetch_xb(sb, fine=False):
            groups = []
            sc = slice(sb * 512, (sb + 1) * 512)
            for g in range(4):
                t = xs.tile([128, 4 * 512], BF16, tag="xg", name="xg")
                tv = t[:].rearrange("p (c s) -> p c s", s=512)
                if fine and g == 0:
                    for c in range(4):
                        nc.sync.dma_start(tv[:, c, :], xb_v[:, c, sc])
                else:
                    nc.sync.dma_start(tv[:, :, :],
                                      xb_v[:, 4 * g:4 * (g + 1), sc])
                groups.append(tv)
            return groups

        def fetch_x8(sb):
            sc = slice(sb * 512, (sb + 1) * 512)
            halves = []
            for h in range(2):
                t = x8s.tile([128, 4 * 2 * 512], FP8, tag="x8h", name="x8h")
                tv = t[:].rearrange("p (cp i s) -> p cp i s", cp=4, i=2)
                nc.sync.dma_start(tv[:, :, :, :],
                                  x8_v[:, 4 * h:4 * (h + 1), :, sc])
                halves.append(tv)
            return halves

        def fetch_tabs(sb):
            sc = slice(sb * 512, (sb + 1) * 512)
            ct = tabs.tile([128, 512], BF16, tag="ctab", name="ctab_t")
            nc.sync.dma_start(ct[:], ctab[:, sc])
            st = tabs.tile([128, 512], BF16, tag="stab", name="stab_t")
            nc.sync.dma_start(st[:], stab[:, sc])
            return ct, st

        # startup: interleave wc/xb so matmul chunk c is fed in order
        sc0 = slice(0, 512)
        t0 = xs.tile([128, 4 * 512], BF16, tag="xg", name="xg")
        t0v = t0[:].rearrange("p (c s) -> p c s", s=512)
        xb_cur = [t0v]
        for c in range(4):
            nc.sync.dma_start(wcv[:, c, :], wc_v[:, c, :])
            nc.sync.dma_start(t0v[:, c, :], xb_v[:, c, sc0])
        for g in range(1, 4):
            t = xs.tile([128, 4 * 512], BF16, tag="xg", name="xg")
            tv = t[:].rearrange("p (c s) -> p c s", s=512)
            nc.sync.dma_start(wcv[:, 4 * g:4 * (g + 1), :],
                              wc_v[:, 4 * g:4 * (g + 1), :])
            nc.sync.dma_start(tv[:, :, :],
                              xb_v[:, 4 * g:4 * (g + 1), sc0])
            xb_cur.append(tv)
        tab_cur = fetch_tabs(0)
        # bf16 q/k weights for the block-0 path (consumed right after
        # the c features), streamed per chunk
        wqkb_tiles = []
        for c in range(16):
            t = wqs.tile([128, 1024], BF16, tag="wq", name="wq")
            nc.sync.dma_start(t[:], wqkb_v[:, c, :])
            wqkb_tiles.append(t)
        x8_cur = fetch_x8(0)
        # w8 per feature tile, rope tiles first (consumed first)
        for ft in (4, 5, 6, 7, 0, 1, 2, 3):
            fs = slice(ft * 2048, (ft + 1) * 2048)
            nc.sync.dma_start(w8_t[:, fs], w8[:, fs])
        nc.sync.dma_start(ones_t[:], onesd[:])
        nc.sync.dma_start(mask_t[:], maskd[:])
        nc.sync.dma_start(eps_t[:], epsd[:])
        nc.sync.dma_start(wuk8_t[:], wuk8[:])
        nc.sync.dma_start(wukbv[:, :, :], wukb_v[:, :, :])
        nc.sync.dma_start(wuvv[:, :, :], wuv_v[:, :, :])
        for j in range(HPG):
            nc.sync.dma_start(wo_t[j][:], wo[j * 128:(j + 1) * 128, :])
        # zero the unused 64-row halves of the rope planes (i=1)
        for j in range(HPG):
            zr = slice(64, 128) if j % 2 == 0 else slice(0, 64)
            nc.sync.dma_start(
                qpv[j][zr, :, 1, :],
                zed[zr, 0:3 * 512].rearrange("p (b s) -> p b s", b=3))
            nc.sync.dma_start(
                kpv[j][zr, :, 1, :],
                zed[zr, :].rearrange("p (b s) -> p b s", b=NT))


        psg = tc.alloc_tile_pool(name="psg", bufs=8, space="PSUM")
        cts = tc.alloc_tile_pool(name="cts", bufs=1)
        crp = tc.alloc_tile_pool(name="crp", bufs=1)
        c8s = tc.alloc_tile_pool(name="c8s", bufs=1)
        rope = tc.alloc_tile_pool(name="rope", bufs=1)
        sqp = tc.alloc_tile_pool(name="sqp", bufs=1)
        rsp = tc.alloc_tile_pool(name="rsp", bufs=1)
        b0p = tc.alloc_tile_pool(name="b0p", bufs=1)
        s3t = tc.alloc_tile_pool(name="s3t", bufs=5)
        s3r = tc.alloc_tile_pool(name="s3r", bufs=2)
        ctxp = tc.alloc_tile_pool(name="ctxp", bufs=1)
        orp = tc.alloc_tile_pool(name="orp", bufs=4)

        def halfswap_dma(dst, src):
            # rope rotate-half: 32-row block swap within each 64 rows
            for blk in range(4):
                sb32 = (blk ^ 1) * 32
                nc.scalar.dma_start(dst[blk * 32:(blk + 1) * 32, :],
                                    src[sb32:sb32 + 32, :])

        for sb in range(SB):
            sc = slice(sb * 512, (sb + 1) * 512)
            qb = sb - 1  # q-pack block index (sb>=1)

            # ---- s1: c features (bf16), per-chunk streaming ----------
            caccs = []
            for f in range(4):
                pt = psg.tile([128, 512], FP32, tag="acc", name=f"cacc{f}")
                caccs.append(pt)
            for c in range(16):
                for f in range(4):
                    nc.tensor.matmul(
                        caccs[f][:], wcv[:, c, f * 128:(f + 1) * 128],
                        xb_cur[c // 4][:, c % 4, :],
                        start=(c == 0), stop=(c == 15),
                    )
            # evacuate craw to SBUF (bf16 = final value precision) so the
            # PSUM banks free immediately; rmsnorm runs off SBUF
            craw = []
            sqs = []
            for f in range(4):
                cr = crp.tile([128, 512], BF16, tag=f"cr{f}", name=f"cr{f}")
                if f % 2 == 0:
                    nc.scalar.copy(cr[:], caccs[f][:])
                else:
                    nc.vector.tensor_copy(cr[:], caccs[f][:])
                craw.append(cr)
                sq = sqp.tile([128, 512], BF16, tag=f"sq{f}", name=f"sq{f}")
                nc.scalar.square(sq[:], caccs[f][:])
                sqs.append(sq)

            ct_sb, st_sb = tab_cur

            def b0_proj():
                qcb, qrotb, krotb = [], [], []
                bacc_ = [psg.tile([128, 512], FP32, tag="acc",
                                  name=f"qb{f}") for f in range(8)]
                for c in range(16):
                    for f in range(8):
                        nc.tensor.matmul(
                            bacc_[f][:],
                            wqkb_tiles[c][:, f * 128:(f + 1) * 128],
                            xb_cur[c // 4][:, c % 4, :],
                            start=(c == 0), stop=(c == 15),
                        )
                for f in range(4):
                    t = b0p.tile([128, 512], BF16, tag=f"qcb{f}",
                                 name=f"qcb{f}")
                    if f % 2 == 0:
                        nc.scalar.copy(t[:], bacc_[f][:])
                    else:
                        nc.vector.tensor_copy(t[:], bacc_[f][:])
                    qcb.append(t)
                for t in range(4):
                    raw = rope.tile([128, 512], BF16, tag=f"rr{t}",
                                    name=f"b0r{t}")
                    if t % 2 == 0:
                        nc.scalar.copy(raw[:], bacc_[4 + t][:])
                    else:
                        nc.vector.tensor_copy(raw[:], bacc_[4 + t][:])
                    swp = rope.tile([128, 512], BF16, tag=f"rw{t}",
                                    name=f"b0w{t}")
                    halfswap_dma(swp, raw)
                    t1 = rope.tile([128, 512], BF16, tag=f"t1{t}",
                                   name=f"b0m{t}")
                    nc.gpsimd.tensor_tensor(out=t1[:], in0=raw[:],
                                            in1=ct_sb[:],
                                            op=mybir.AluOpType.mult)
                    t2 = rope.tile([128, 512], BF16, tag=f"t2{t}",
                                   name=f"b0n{t}")
                    nc.gpsimd.tensor_tensor(out=t2[:], in0=swp[:],
                                            in1=st_sb[:],
                                            op=mybir.AluOpType.mult)
                    rot = b0p.tile([128, 512], BF16, tag=f"b0o{t}",
                                   name=f"b0o{t}")
                    nc.gpsimd.tensor_tensor(out=rot[:], in0=t1[:],
                                            in1=t2[:],
                                            op=mybir.AluOpType.add)
                    (qrotb if t < 2 else krotb).append(rot)
                return qcb, qrotb, krotb

            # ---- s1 qk features (fp8 DoubleRow) ----------------------
            # features: 0-3 qc, 4-5 q-rope raw, 6-7 k-rope raw.
            # At sb=0 the q features are skipped (bf16 path above).
            rope_mul = {}

            def qk_feature(f):
                pt = psg.tile([128, 512], FP32, tag="acc", name=f"qk{f}")
                for cp in range(8):
                    nc.tensor.matmul(
                        pt[:], w8v[:, f, cp, :, :],
                        x8_cur[cp // 4][:, cp % 4, :, :],
                        start=(cp == 0), stop=(cp == 7),
                        perf_mode=DR,
                    )
                if f < 4:
                    nc.vector.tensor_copy(qpv[f][:, qb, 0, :], pt[:])
                    return
                t = f - 4
                rr = rope.tile([128, 512], BF16, tag=f"rr{t}", name=f"rr{t}")
                nc.scalar.copy(rr[:], pt[:])
                rw = rope.tile([128, 512], BF16, tag=f"rw{t}", name=f"rw{t}")
                halfswap_dma(rw, rr)
                t1 = rope.tile([128, 512], BF16, tag=f"t1{t}", name=f"t1{t}")
                nc.gpsimd.tensor_tensor(out=t1[:], in0=rr[:], in1=ct_sb[:],
                                        op=mybir.AluOpType.mult)
                t2 = rope.tile([128, 512], BF16, tag=f"t2{t}", name=f"t2{t}")
                nc.gpsimd.tensor_tensor(out=t2[:], in0=rw[:], in1=st_sb[:],
                                        op=mybir.AluOpType.mult)
                rope_mul[t] = (t1, t2)

            feats_a = () if sb == 0 else (4, 5, 6, 7, 0)
            feats_b = (6, 7) if sb == 0 else (1, 2, 3)
            for f in feats_a:
                qk_feature(f)

            # rmsnorm: msum -> rs -> rsr -> cT8 (fp8) + cT (bf16)
            msum = psg.tile([128, 512], FP32, tag="acc", name="msum")
            for f in range(4):
                nc.tensor.matmul(msum[:], ones_t[:], sqs[f][:],
                                 start=(f == 0), stop=(f == 3))
            cT = []
            c8v = []
            with tc.high_priority():
                rs = rsp.tile([128, 512], FP32, tag="rs", name="rs")
                nc.scalar.activation(rs[:], msum[:],
                                     mybir.ActivationFunctionType.Sqrt,
                                     bias=eps_t[:], scale=1.0 / L)
                rsr = rsp.tile([128, 512], FP32, tag="rsr", name="rsr")
                nc.vector.reciprocal(rsr[:], rs[:])
                for q in range(2):
                    t8 = c8s.tile([128, 2 * 512], FP8, tag=f"c8{q}",
                                  name=f"c8{q}")
                    c8v.append(t8[:].rearrange("p (i s) -> p i s", i=2))
                for lc in range(4):
                    nc.vector.tensor_tensor(out=c8v[lc // 2][:, lc % 2, :],
                                            in0=craw[lc][:], in1=rsr[:],
                                            op=mybir.AluOpType.mult)
                for lc in range(4):
                    ct = cts.tile([128, 512], BF16, tag=f"cT{lc}",
                                  name=f"cT{lc}")
                    nc.vector.tensor_tensor(out=ct[:], in0=craw[lc][:],
                                            in1=rsr[:],
                                            op=mybir.AluOpType.mult)
                    cT.append(ct)

            if sb == 0:
                qcb, qrotb, krotb = b0_proj()
            for f in feats_b:
                qk_feature(f)

            # rope pack adds (Pool; q-planes skipped at sb=0)
            for t in rope_mul:
                t1, t2 = rope_mul[t]
                for hh in range(2):
                    j = (t % 2) * 2 + hh
                    pr = slice(hh * 64, (hh + 1) * 64)
                    if t < 2:
                        nc.gpsimd.tensor_tensor(
                            out=qpv[j][pr, qb, 1, :],
                            in0=t1[pr, :], in1=t2[pr, :],
                            op=mybir.AluOpType.add)
                    else:
                        nc.gpsimd.tensor_tensor(
                            out=kpv[j][pr, 4 * sb:4 * sb + 4, 1, :],
                            in0=t1[pr, :].rearrange("p (a b) -> p a b",
                                                    b=128),
                            in1=t2[pr, :].rearrange("p (a b) -> p a b",
                                                    b=128),
                            op=mybir.AluOpType.add)

            # prefetch next block's activations while s2/s3 run
            if sb + 1 < SB:
                xb_nxt = fetch_xb(sb + 1)
                x8_nxt = fetch_x8(sb + 1)
                tab_nxt = fetch_tabs(sb + 1)

            # ---- s2: absorption for this token block -----------------
            for j in range(HPG):
                pt = psg.tile([128, 512], FP32, tag="acc", name="kab")
                for q in range(2):
                    nc.tensor.matmul(
                        pt[:], wukv[:, q, j, :, :],
                        c8v[q][:, :, :],
                        start=(q == 0), stop=(q == 1), perf_mode=DR,
                    )
                nc.scalar.copy(
                    kpv[j][:, 4 * sb:4 * sb + 4, 0, :],
                    pt[:].rearrange("p (a b) -> p a b", b=128))
            for tt in range(4):
                pt = psg.tile([128, 512], FP32, tag="acc", name="vab")
                for lc in range(4):
                    nc.tensor.matmul(
                        pt[:], cT[lc][:, tt * 128:(tt + 1) * 128],
                        wuvv[:, lc, :],
                        start=(lc == 0), stop=(lc == 3),
                    )
                nc.vector.tensor_copy(vabs[4 * sb + tt][:], pt[:])

            # ---- block-0 bf16 path, part 2: kabs (needs cT) ----------
            if sb == 0:
                kabs_b = []
                for j in range(HPG):
                    pt = psg.tile([128, 512], FP32, tag="acc", name="kabb")
                    for lc in range(4):
                        nc.tensor.matmul(
                            pt[:],
                            wukbv[:, lc, j * 128:(j + 1) * 128],
                            cT[lc][:],
                            start=(lc == 0), stop=(lc == 3),
                        )
                    t = b0p.tile([128, 512], BF16, tag=f"kabb{j}",
                                 name=f"kabb{j}")
                    nc.scalar.copy(t[:], pt[:])
                    kabs_b.append(t)

            # ---- s3: attention for q-block sb ------------------------
            nti = 4 * sb + 4
            LAG = 2

            def s3_make(jpair):
                ja, jb = 2 * jpair, 2 * jpair + 1
                return {
                    "ja": ja, "jb": jb, "pfs": {}, "emitted": 0,
                    "cx": [psg.tile([128, 512], FP32, tag="acc",
                                    name=f"cx{j}") for j in (ja, jb)],
                    "sm": [psg.tile([128, 512], FP32, tag="acc",
                                    name=f"sm{j}") for j in (ja, jb)],
                }

            def s3_scores(st, ti):
                ja, jb = st["ja"], st["jb"]
                j0 = ti - 4 * sb
                c0 = max(j0, 0) * 128
                pfpair = []
                for j in (ja, jb):
                    scp = psg.tile([128, 512], FP32, tag="acc", name="sc")
                    if sb == 0:
                        tsl = slice(ti * 128, (ti + 1) * 128)
                        rsl = slice((j % 2) * 64, (j % 2) * 64 + 64)
                        nc.tensor.matmul(
                            scp[:, c0:512],
                            kabs_b[j][:, tsl], qcb[j][:, c0:512],
                            start=True, stop=False,
                        )
                        nc.tensor.matmul(
                            scp[:, c0:512],
                            krotb[j // 2][rsl, tsl],
                            qrotb[j // 2][rsl, c0:512],
                            start=False, stop=True,
                        )
                    else:
                        nc.tensor.matmul(
                            scp[:],
                            kpv[j][:, ti, :, :],
                            qpv[j][:, qb, :, :],
                            start=True, stop=True, perf_mode=DR,
                        )
                    pf = s3t.tile([128, 512], BF16, tag="pf", name="pf")
                    nc.scalar.activation(pf[:, c0:512], scp[:, c0:512],
                                         mybir.ActivationFunctionType.Exp,
                                         scale=SCALE)
                    if j0 >= 0:
                        nc.vector.tensor_tensor(
                            out=pf[:, c0:c0 + 128], in0=pf[:, c0:c0 + 128],
                            in1=mask_t[:], op=mybir.AluOpType.mult)
                    pfpair.append(pf)
                st["pfs"][ti] = (pfpair, c0)

            def s3_pv(st, tv):
                ja, jb = st["ja"], st["jb"]
                (pfa, pfb), c0 = st["pfs"].pop(tv)
                for (pf, cxp, smp, j) in ((pfa, st["cx"][0], st["sm"][0], ja),
                                          (pfb, st["cx"][1], st["sm"][1], jb)):
                    nc.tensor.matmul(
                        cxp[:, c0:512],
                        vabs[tv][:, j * 128:(j + 1) * 128],
                        pf[:, c0:512],
                        start=(tv == 0), stop=(tv == nti - 1),
                    )
                    nc.tensor.matmul(
                        smp[:, c0:512], ones_t[:], pf[:, c0:512],
                        start=(tv == 0), stop=(tv == nti - 1),
                    )

            def s3_run(st, tis):
                for ti in tis:
                    s3_scores(st, ti)
                    st["emitted"] += 1
                    if st["emitted"] > LAG:
                        s3_pv(st, ti - LAG)

            def s3_finish(st):
                for tv in sorted(st["pfs"].keys()):
                    s3_pv(st, tv)
                ja, jb = st["ja"], st["jb"]
                outs = []
                for i, j in enumerate((ja, jb)):
                    rec = s3r.tile([128, 512], FP32, tag="rec", name="rec")
                    nc.vector.reciprocal(rec[:], st["sm"][i][:])
                    ctx = ctxp.tile([128, 512], BF16, tag=f"ctx{j}",
                                    name=f"ctx{j}")
                    nc.vector.tensor_tensor(out=ctx[:], in0=st["cx"][i][:],
                                            in1=rec[:],
                                            op=mybir.AluOpType.mult)
                    outs.append(ctx)
                return outs

            stA = s3_make(0)
            s3_run(stA, range(nti))
            ctxA = s3_finish(stA)
            stB = s3_make(1)
            s3_run(stB, range(nti))
            ctxB = s3_finish(stB)
            ctx = ctxA + ctxB

            # ---- s4: output projection for this s block --------------
            for dt in range(16):
                dc = slice(dt * 128, (dt + 1) * 128)
                pt = psg.tile([128, 512], FP32, tag="acc", name="po")
                for j in range(HPG):
                    nc.tensor.matmul(pt[:], wo_t[j][:, dc], ctx[j][:],
                                     start=(j == 0), stop=(j == HPG - 1))
                orow = orp.tile([128, 512], BF16, tag="orow", name="orow")
                if sb < SB - 1 and dt % 2 == 0:
                    nc.scalar.copy(orow[:], pt[:])
                else:
                    nc.vector.tensor_copy(orow[:], pt[:])
                nc.sync.dma_start(outT[dc, sc], orow[:])

            if sb + 1 < SB:
                xb_cur, x8_cur, tab_cur = xb_nxt, x8_nxt, tab_nxt

        orp.release()
        ctxp.release()
        s3r.release()
        s3t.release()
        b0p.release()
        rsp.release()
        sqp.release()
        rope.release()
        c8s.release()
        crp.release()
        cts.release()
        psg.release()
        wqs.release()
        tabs.release()
        x8s.release()
        xs.release()
        vp.release()
        packs.release()
        wgt.release()
        cst.release()

    nc.compile()
    return nc


def _host_inputs(core, x, W_DKV, W_KRope, W_Q, W_UK, W_UV, W_O, kv_norm_w,
                 offset):
    b, g = core // HPG, core % HPG
    F8 = ml_dtypes.float8_e4m3
    BF = ml_dtypes.bfloat16

    xT = np.ascontiguousarray(x[b].T)  # [DIN, S] fp32
    x8 = np.ascontiguousarray(
        xT.reshape(8, 2, 128, S).transpose(2, 0, 1, 3)
    ).astype(F8).reshape(128, -1)

    qc_rows = W_Q[512 * g:512 * (g + 1)]
    qr_rows = W_Q[DOUT + 256 * g:DOUT + 256 * (g + 1)]
    kr_rows = W_KRope[256 * g:256 * (g + 1)]
    wqk = np.concatenate([qc_rows, qr_rows, kr_rows], axis=0)  # [1024, DIN]
    # [p, ftile, cp, i, f] with k = cp*256 + i*128 + p
    w8 = np.ascontiguousarray(
        wqk.T.reshape(8, 2, 128, 8, 128).transpose(2, 3, 0, 1, 4)
    ).astype(F8).reshape(128, -1)

    wukf = (W_UK[512 * g:512 * (g + 1)] * kv_norm_w[None, :]).T  # [L, 512]
    # [p, q, j, i, d] with l = q*256 + i*128 + p, col = j*128 + d
    wuk8 = np.ascontiguousarray(
        wukf.reshape(2, 2, 128, 4, 128).transpose(2, 0, 3, 1, 4)
    ).astype(F8).reshape(128, -1)
    wuvf = (W_UV[512 * g:512 * (g + 1)] * kv_norm_w[None, :]).T

    pos = (float(offset) + np.arange(S, dtype=np.float64))
    inv = 1.0 / ROPE_BASE ** (np.arange(0, RD, 2, dtype=np.float64) / RD)
    ang = pos[:, None] * inv[None, :]               # [S, 32]
    cos64 = np.concatenate([np.cos(ang), np.cos(ang)], axis=1).T  # [64,S]
    st64 = np.concatenate([-np.sin(ang), np.sin(ang)], axis=1).T
    ctab = np.concatenate([cos64, cos64], axis=0).astype(BF)
    stab = np.concatenate([st64, st64], axis=0).astype(BF)

    return {
        "xb": xT.astype(BF),
        "x8": x8,
        "wc": np.ascontiguousarray(W_DKV.T.astype(BF)),
        "w8": w8,
        "wqkb": np.ascontiguousarray(wqk.T.astype(BF)),
        "wuk8": wuk8,
        "wukb": np.ascontiguousarray(wukf.astype(BF)),
        "wuv": np.ascontiguousarray(wuvf.astype(BF)),
        "wo": np.ascontiguousarray(W_O[:, 512 * g:512 * (g + 1)].T.astype(BF)),
        "ctab": ctab,
        "stab": stab,
        "onesd": np.ones((128, 128), dtype=BF),
        "maskd": np.triu(np.ones((128, 128))).astype(BF),
        "epsd": np.full((128, 1), EPS, dtype=np.float32),
        "zed": np.zeros((128, S), dtype=F8),
    }


def _reference_numpy(x, W_DKV, W_KRope, W_Q, W_UK, W_UV, W_O, kv_norm_w,
                     offset):
    """Pure-numpy fallback (only used if offset != 0)."""
    x = x.astype(np.float64)
    b, s, _ = x.shape
    ckv = x @ W_DKV.T.astype(np.float64)
    ms = np.mean(ckv * ckv, axis=-1, keepdims=True)
    ckv = ckv / np.sqrt(ms + EPS) * kv_norm_w
    kr = (x @ W_KRope.T.astype(np.float64)).reshape(b, s, H, RD).transpose(0, 2, 1, 3)
    qt = x @ W_Q.T.astype(np.float64)
    qc = qt[..., :DOUT].reshape(b, s, H, HD).transpose(0, 2, 1, 3)
    qr = qt[..., DOUT:].reshape(b, s, H, RD).transpose(0, 2, 1, 3)

    def rope(t):
        pos = float(offset) + np.arange(s)
        inv = 1.0 / ROPE_BASE ** (np.arange(0, RD, 2) / RD)
        ang = pos[:, None] * inv[None, :]
        cos = np.concatenate([np.cos(ang)] * 2, -1)
        sin = np.concatenate([np.sin(ang)] * 2, -1)
        t1, t2 = t[..., :RD // 2], t[..., RD // 2:]
        rot = np.concatenate([-t2, t1], -1)
        return t * cos + rot * sin

    qr, kr = rope(qr), rope(kr)
    wuk = W_UK.reshape(H, HD, L).astype(np.float64)
    qa = np.einsum("bhsd,hdl->bhsl", qc, wuk)
    sc = (np.einsum("bhsl,btl->bhst", qa, ckv)
          + np.einsum("bhsr,bhtr->bhst", qr, kr)) / math.sqrt(HD + RD)
    causal = np.arange(s)[None, :] > (np.arange(s)[:, None] + int(offset))
    sc = np.where(causal[None, None], -np.inf, sc)
    sc = sc - sc.max(-1, keepdims=True)
    e = np.exp(sc)
    attn = e / e.sum(-1, keepdims=True)
    cl = np.einsum("bhst,btl->bhsl", attn, ckv)
    wuv = W_UV.reshape(H, HD, L).astype(np.float64)
    ctx = np.einsum("bhsl,hdl->bhsd", cl, wuv)
    ctx = ctx.transpose(0, 2, 1, 3).reshape(b, s, DOUT)
    return (ctx @ W_O.T.astype(np.float64)).astype(np.float32)


def kernel(**inputs) -> np.ndarray:
    args = {k: np.asarray(v) if not np.isscalar(v) else v
            for k, v in inputs.items()}
    x = np.asarray(args["x"], dtype=np.float32)
    offset = int(np.asarray(args["offset"]))
    names = ["x", "W_DKV", "W_KRope", "W_Q", "W_UK", "W_UV", "W_O",
             "kv_norm_w"]
    arrs = {n: np.asarray(args[n], dtype=np.float32) for n in names}

    if offset != 0:
        return _reference_numpy(**arrs, offset=offset)

    if "nc" not in _cache:
        _cache["nc"] = _build()
    nc = _cache["nc"]

    from concourse.bass_utils import run_bass_kernel_spmd

    in_maps = [
        _host_inputs(core, offset=offset, **arrs) for core in range(NCORES)
    ]
    res = run_bass_kernel_spmd(nc, in_maps, list(range(NCORES)))
    _cache["last_result"] = res

    out = np.zeros((B, S, DIN), dtype=np.float32)
    for core in range(NCORES):
        b = core // HPG
        out[b] += res.results[core]["outT"].T.astype(np.float32)
    return out


# revision 61
# speedup vs baseline: 1.3694x; 1.0753x over previous
"""DeepSeekV3 latent attention (MLA) on 8 TRN2 NeuronCores via Bass/Tile.

Sharding: 8 cores = 2 batches x 4 head-groups (4 heads each). Each core
runs the full pipeline for its batch/heads and emits a W_O row-slice
partial output; the host sums the 4 partials per batch (bf16 outT).

Numerics: the value path (c_kv latent used as V, W_UV absorption, ctx,
W_O) is bf16 - its error lands directly on the output.  The query/key
path (q/k/rope projections, W_UK absorption, attention logits) is
fp8e4m3 run in DoubleRow perf mode (K=256 contractions at 0.5 cyc/row):
logit noise averages away under softmax over many tokens.  Queries in
the FIRST s-block attend few tokens (no averaging), so q-block 0 gets a
dedicated bf16 attention path (bf16 q/k projections + bf16 scores);
blocks 1-3 use fp8 packs that fuse content (128) + rope (64) into ONE
DoubleRow matmul per token tile.  DoubleRow operand slices must be
contiguous ([p, 2, N]) - strided plane slices lock up the PE - so the
q pack is blocked per s-block and the k pack per token tile.

Layout is feature-major throughout ([feat, s] / [latent, t]); rope
uses host cos/sin tables plus an SBUF-to-SBUF 32-row half-swap DMA.
Per s-block the kernel fuses: projections -> rmsnorm -> K/V absorption
-> causal attention for this q-block -> W_O partial -> DMA out.
"""

import math
import ml_dtypes
import numpy as np

import concourse.bacc as bacc
import concourse.tile as tile
from concourse import mybir

FP32 = mybir.dt.float32
BF16 = mybir.dt.bfloat16
FP8 = mybir.dt.float8e4
DR = mybir.MatmulPerfMode.DoubleRow

B, S, DIN = 2, 2048, 2048
DOUT, H, RD, L = 2048, 16, 64, 512
HD = DOUT // H  # 128
ROPE_BASE = 10000.0
EPS = 1e-6
HPG = 4          # heads per group (per core)
NCORES = 8
SB = 4           # s blocks of 512
NT = 16          # token tiles of 128
SCALE = 1.0 / math.sqrt(HD + RD)

_cache = {}


def _build():
    nc = bacc.Bacc("TRN2", target_bir_lowering=False, debug=False)

    xb = nc.dram_tensor("xb", [DIN, S], BF16, kind="ExternalInput").ap()
    x8 = nc.dram_tensor("x8", [128, 8 * 2 * S], FP8, kind="ExternalInput").ap()
    wc = nc.dram_tensor("wc", [DIN, 512], BF16, kind="ExternalInput").ap()
    w8 = nc.dram_tensor("w8", [128, 8 * 2048], FP8, kind="ExternalInput").ap()
    wqkb = nc.dram_tensor("wqkb", [DIN, 1024], BF16,
                          kind="ExternalInput").ap()
    wuk8 = nc.dram_tensor("wuk8", [128, 2 * 4 * 2 * 128], FP8,
                          kind="ExternalInput").ap()
    wukb = nc.dram_tensor("wukb", [L, 512], BF16, kind="ExternalInput").ap()
    wuv = nc.dram_tensor("wuv", [L, 512], BF16, kind="ExternalInput").ap()
    wo = nc.dram_tensor("wo", [512, DIN], BF16, kind="ExternalInput").ap()
    ctab = nc.dram_tensor("ctab", [128, S], BF16, kind="ExternalInput").ap()
    stab = nc.dram_tensor("stab", [128, S], BF16, kind="ExternalInput").ap()
    onesd = nc.dram_tensor("onesd", [128, 128], BF16, kind="ExternalInput").ap()
    maskd = nc.dram_tensor("maskd", [128, 128], BF16, kind="ExternalInput").ap()
    epsd = nc.dram_tensor("epsd", [128, 1], FP32, kind="ExternalInput").ap()
    zed = nc.dram_tensor("zed", [128, S], FP8, kind="ExternalInput").ap()
    outT = nc.dram_tensor("outT", [DIN, S], BF16, kind="ExternalOutput").ap()

    xb_v = xb.rearrange("(c p) s -> p c s", p=128)          # [128,16,S]
    x8_v = x8.rearrange("p (cp i s) -> p cp i s", cp=8, i=2)
    wc_v = wc.rearrange("(c p) f -> p c f", p=128)          # [128,16,512]
    # feature-tile-major so per-feature DMAs are contiguous
    w8_v = w8.rearrange("p (ft cp i f) -> p ft cp i f", ft=8, cp=8, i=2)
    wqkb_v = wqkb.rearrange("(c p) f -> p c f", p=128)      # [128,16,1024]
    wukb_v = wukb.rearrange("(lc p) d -> p lc d", p=128)
    wuv_v = wuv.rearrange("(lc p) d -> p lc d", p=128)      # [128,4,512]

    with tile.TileContext(nc) as tc:
        cst = tc.alloc_tile_pool(name="cst", bufs=1)
        ones_t = cst.tile([128, 128], BF16, tag="ones", name="ones_t")
        mask_t = cst.tile([128, 128], BF16, tag="mask", name="mask_t")
        eps_t = cst.tile([128, 1], FP32, tag="eps", name="eps_t")

        wgt = tc.alloc_tile_pool(name="wgt", bufs=1, side="right")
        wc_t = wgt.tile([128, 16 * 512], BF16, tag="wc", name="wc_t")
        w8_t = wgt.tile([128, 8 * 2048], FP8, tag="w8", name="w8_t")
        wuk8_t = wgt.tile([128, 2048], FP8, tag="wuk8", name="wuk8_t")
        wukb_t = wgt.tile([128, 4 * 512], BF16, tag="wukb", name="wukb_t")
        wuv_t = wgt.tile([128, 4 * 512], BF16, tag="wuv", name="wuv_t")
        wo_t = [wgt.tile([128, DIN], BF16, tag=f"wo{j}", name=f"wo{j}")
                for j in range(HPG)]
        wcv = wc_t[:].rearrange("p (c f) -> p c f", f=512)
        w8v = w8_t[:].rearrange("p (ft cp i f) -> p ft cp i f",
                                ft=8, cp=8, i=2)
        wukv = wuk8_t[:].rearrange("p (q j i d) -> p q j i d",
                                   q=2, j=4, i=2)
        wukbv = wukb_t[:].rearrange("p (lc d) -> p lc d", d=512)
        wuvv = wuv_t[:].rearrange("p (lc d) -> p lc d", d=512)

        packs = tc.alloc_tile_pool(name="packs", bufs=1)
        # qp: [p, qb(3: s-blocks 1..3), i, 512]; kp: [p, tt, i, 128].
        # DoubleRow slices [:, blk, :, :] are contiguous.
        qp = [packs.tile([128, 3 * 2 * 512], FP8, tag=f"qp{j}",
                         name=f"qp{j}") for j in range(HPG)]
        kp = [packs.tile([128, NT * 2 * 128], FP8, tag=f"kp{j}",
                         name=f"kp{j}") for j in range(HPG)]
        qpv = [t[:].rearrange("p (b i s) -> p b i s", b=3, i=2)
               for t in qp]
        kpv = [t[:].rearrange("p (b i s) -> p b i s", b=NT, i=2)
               for t in kp]

        vp = tc.alloc_tile_pool(name="vp", bufs=1)
        vabs = [vp.tile([128, 512], BF16, tag=f"vabs{t}", name=f"vabs{t}")
                for t in range(NT)]

        # ---- initial DMAs (all HWDGE via sync; Pool stays free) -----
        xs = tc.alloc_tile_pool(name="xs", bufs=5)
        x8s = tc.alloc_tile_pool(name="x8s", bufs=3)
        tabs = tc.alloc_tile_pool(name="tabs", bufs=1)
        wqs = tc.alloc_tile_pool(name="wqs", bufs=3)

        def fetch_xb(sb, fine=False):
            groups = []
            sc = slice(sb * 512, (sb + 1) * 512)
            for g in range(4):
                t = xs.tile([128, 4 * 512], BF16, tag="xg", name="xg")
                tv = t[:].rearrange("p (c s) -> p c s", s=512)
                if fine and g == 0:
                    for c in range(4):
                        nc.sync.dma_start(tv[:, c, :], xb_v[:, c, sc])
                else:
                    nc.sync.dma_start(tv[:, :, :],
                                      xb_v[:, 4 * g:4 * (g + 1), sc])
                groups.append(tv)
            return groups

        def fetch_x8(sb):
            sc = slice(sb * 512, (sb + 1) * 512)
            halves = []
            for h in range(2):
                t = x8s.tile([128, 4 * 2 * 512], FP8, tag="x8h", name="x8h")
                tv = t[:].rearrange("p (cp i s) -> p cp i s", cp=4, i=2)
                nc.sync.dma_start(tv[:, :, :, :],
                                  x8_v[:, 4 * h:4 * (h + 1), :, sc])
                halves.append(tv)
            return halves

        def fetch_tabs(sb):
            sc = slice(sb * 512, (sb + 1) * 512)
            ct = tabs.tile([128, 512], BF16, tag="ctab", name="ctab_t")
            nc.sync.dma_start(ct[:], ctab[:, sc])
            st = tabs.tile([128, 512], BF16, tag="stab", name="stab_t")
            nc.sync.dma_start(st[:], stab[:, sc])
            return ct, st

        # startup: interleave wc/xb so matmul chunk c is fed in order
        sc0 = slice(0, 512)
        t0 = xs.tile([128, 4 * 512], BF16, tag="xg", name="xg")
        t0v = t0[:].rearrange("p (c s) -> p c s", s=512)
        xb_cur = [t0v]
        for c in range(4):
            nc.sync.dma_start(wcv[:, c, :], wc_v[:, c, :])
            nc.sync.dma_start(t0v[:, c, :], xb_v[:, c, sc0])
        for g in range(1, 4):
            t = xs.tile([128, 4 * 512], BF16, tag="xg", name="xg")
            tv = t[:].rearrange("p (c s) -> p c s", s=512)
            nc.sync.dma_start(wcv[:, 4 * g:4 * (g + 1), :],
                              wc_v[:, 4 * g:4 * (g + 1), :])
            nc.sync.dma_start(tv[:, :, :],
                              xb_v[:, 4 * g:4 * (g + 1), sc0])
            xb_cur.append(tv)
        tab_cur = fetch_tabs(0)
        # bf16 q/k weights for the block-0 path (consumed right after
        # the c features), streamed per chunk
        wqkb_tiles = []
        for c in range(16):
            t = wqs.tile([128, 1024], BF16, tag="wq", name="wq")
            nc.sync.dma_start(t[:], wqkb_v[:, c, :])
            wqkb_tiles.append(t)
        nc.sync.dma_start(ones_t[:], onesd[:])
        nc.sync.dma_start(mask_t[:], maskd[:])
        nc.sync.dma_start(eps_t[:], epsd[:])
        x8_cur = fetch_x8(0)
        # w8 per feature tile, rope tiles first (consumed first)
        for ft in (4, 5, 6, 7, 0, 1, 2, 3):
            fs = slice(ft * 2048, (ft + 1) * 2048)
            nc.sync.dma_start(w8_t[:, fs], w8[:, fs])
        nc.sync.dma_start(wuk8_t[:], wuk8[:])
        nc.sync.dma_start(wukbv[:, :, :], wukb_v[:, :, :])
        nc.sync.dma_start(wuvv[:, :, :], wuv_v[:, :, :])
        for j in range(HPG):
            nc.sync.dma_start(wo_t[j][:], wo[j * 128:(j + 1) * 128, :])
        # zero the unused 64-row halves of the rope planes (i=1)
        for j in range(HPG):
            zr = slice(64, 128) if j % 2 == 0 else slice(0, 64)
            nc.sync.dma_start(
                qpv[j][zr, :, 1, :],
                zed[zr, 0:3 * 512].rearrange("p (b s) -> p b s", b=3))
            nc.sync.dma_start(
                kpv[j][zr, :, 1, :],
                zed[zr, :].rearrange("p (b s) -> p b s", b=NT))


        psg = tc.alloc_tile_pool(name="psg", bufs=8, space="PSUM")
        cts = tc.alloc_tile_pool(name="cts", bufs=1)
        crp = tc.alloc_tile_pool(name="crp", bufs=1)
        c8s = tc.alloc_tile_pool(name="c8s", bufs=1)
        rope = tc.alloc_tile_pool(name="rope", bufs=1)
        sqp = tc.alloc_tile_pool(name="sqp", bufs=1)
        rsp = tc.alloc_tile_pool(name="rsp", bufs=1)
        b0p = tc.alloc_tile_pool(name="b0p", bufs=1)
        s3t = tc.alloc_tile_pool(name="s3t", bufs=5)
        s3r = tc.alloc_tile_pool(name="s3r", bufs=2)
        ctxp = tc.alloc_tile_pool(name="ctxp", bufs=1)
        orp = tc.alloc_tile_pool(name="orp", bufs=4)

        def halfswap_dma(dst, src):
            # rope rotate-half: 32-row block swap within each 64 rows
            for blk in range(4):
                sb32 = (blk ^ 1) * 32
                nc.scalar.dma_start(dst[blk * 32:(blk + 1) * 32, :],
                                    src[sb32:sb32 + 32, :])

        for sb in range(SB):
            sc = slice(sb * 512, (sb + 1) * 512)
            qb = sb - 1  # q-pack block index (sb>=1)

            # ---- s1: c features (bf16), per-chunk streaming ----------
            caccs = []
            for f in range(4):
                pt = psg.tile([128, 512], FP32, tag="acc", name=f"cacc{f}")
                caccs.append(pt)
            for c in range(16):
                for f in range(4):
                    nc.tensor.matmul(
                        caccs[f][:], wcv[:, c, f * 128:(f + 1) * 128],
                        xb_cur[c // 4][:, c % 4, :],
                        start=(c == 0), stop=(c == 15),
                    )
            # evacuate craw to SBUF (bf16 = final value precision) so the
            # PSUM banks free immediately; rmsnorm runs off SBUF
            craw = []
            sqs = []
            for f in range(4):
                cr = crp.tile([128, 512], BF16, tag=f"cr{f}", name=f"cr{f}")
                if f % 2 == 0:
                    nc.scalar.copy(cr[:], caccs[f][:])
                else:
                    nc.vector.tensor_copy(cr[:], caccs[f][:])
                craw.append(cr)
                sq = sqp.tile([128, 512], BF16, tag=f"sq{f}", name=f"sq{f}")
                nc.vector.tensor_tensor(out=sq[:], in0=cr[:], in1=cr[:],
                                        op=mybir.AluOpType.mult)
                sqs.append(sq)

            ct_sb, st_sb = tab_cur

            def b0_proj():
                qcb, qrotb, krotb = [], [], []
                bacc_ = [psg.tile([128, 512], FP32, tag="acc",
                                  name=f"qb{f}") for f in range(8)]
                for c in range(16):
                    for f in range(8):
                        nc.tensor.matmul(
                            bacc_[f][:],
                            wqkb_tiles[c][:, f * 128:(f + 1) * 128],
                            xb_cur[c // 4][:, c % 4, :],
                            start=(c == 0), stop=(c == 15),
                        )
                for f in range(4):
                    t = b0p.tile([128, 512], BF16, tag=f"qcb{f}",
                                 name=f"qcb{f}")
                    nc.vector.tensor_copy(t[:], bacc_[f][:])
                    qcb.append(t)
                for t in range(4):
                    raw = rope.tile([128, 512], BF16, tag=f"rr{t}",
                                    name=f"b0r{t}")
                    nc.scalar.copy(raw[:], bacc_[4 + t][:])
                    swp = rope.tile([128, 512], BF16, tag=f"rw{t}",
                                    name=f"b0w{t}")
                    halfswap_dma(swp, raw)
                    t1 = rope.tile([128, 512], BF16, tag=f"t1{t}",
                                   name=f"b0m{t}")
                    nc.gpsimd.tensor_tensor(out=t1[:], in0=raw[:],
                                            in1=ct_sb[:],
                                            op=mybir.AluOpType.mult)
                    t2 = rope.tile([128, 512], BF16, tag=f"t2{t}",
                                   name=f"b0n{t}")
                    nc.gpsimd.tensor_tensor(out=t2[:], in0=swp[:],
                                            in1=st_sb[:],
                                            op=mybir.AluOpType.mult)
                    rot = b0p.tile([128, 512], BF16, tag=f"b0o{t}",
                                   name=f"b0o{t}")
                    nc.gpsimd.tensor_tensor(out=rot[:], in0=t1[:],
                                            in1=t2[:],
                                            op=mybir.AluOpType.add)
                    (qrotb if t < 2 else krotb).append(rot)
                return qcb, qrotb, krotb

            # ---- s1 qk features (fp8 DoubleRow) ----------------------
            # features: 0-3 qc, 4-5 q-rope raw, 6-7 k-rope raw.
            # At sb=0 the q features are skipped (bf16 path above).
            rope_mul = {}

            def qk_feature(f):
                pt = psg.tile([128, 512], FP32, tag="acc", name=f"qk{f}")
                for cp in range(8):
                    nc.tensor.matmul(
                        pt[:], w8v[:, f, cp, :, :],
                        x8_cur[cp // 4][:, cp % 4, :, :],
                        start=(cp == 0), stop=(cp == 7),
                        perf_mode=DR,
                    )
                if f < 4:
                    nc.vector.tensor_copy(qpv[f][:, qb, 0, :], pt[:])
                    return
                t = f - 4
                rr = rope.tile([128, 512], BF16, tag=f"rr{t}", name=f"rr{t}")
                nc.scalar.copy(rr[:], pt[:])
                rw = rope.tile([128, 512], BF16, tag=f"rw{t}", name=f"rw{t}")
                halfswap_dma(rw, rr)
                t1 = rope.tile([128, 512], BF16, tag=f"t1{t}", name=f"t1{t}")
                nc.gpsimd.tensor_tensor(out=t1[:], in0=rr[:], in1=ct_sb[:],
                                        op=mybir.AluOpType.mult)
                t2 = rope.tile([128, 512], BF16, tag=f"t2{t}", name=f"t2{t}")
                nc.gpsimd.tensor_tensor(out=t2[:], in0=rw[:], in1=st_sb[:],
                                        op=mybir.AluOpType.mult)
                rope_mul[t] = (t1, t2)

            feats_a = () if sb == 0 else (4, 5, 6, 7)
            feats_b = (6, 7) if sb == 0 else (0, 1, 2, 3)
            for f in feats_a:
                qk_feature(f)

            # rmsnorm: msum -> rs -> rsr -> cT8 (fp8) + cT (bf16)
            msum = psg.tile([128, 512], FP32, tag="acc", name="msum")
            for f in range(4):
                nc.tensor.matmul(msum[:], ones_t[:], sqs[f][:],
                                 start=(f == 0), stop=(f == 3))
            cT = []
            c8v = []
            with tc.high_priority():
                rs = rsp.tile([128, 512], FP32, tag="rs", name="rs")
                nc.scalar.activation(rs[:], msum[:],
                                     mybir.ActivationFunctionType.Sqrt,
                                     bias=eps_t[:], scale=1.0 / L)
                rsr = rsp.tile([128, 512], FP32, tag="rsr", name="rsr")
                nc.vector.reciprocal(rsr[:], rs[:])
                for q in range(2):
                    t8 = c8s.tile([128, 2 * 512], FP8, tag=f"c8{q}",
                                  name=f"c8{q}")
                    c8v.append(t8[:].rearrange("p (i s) -> p i s", i=2))
                for lc in range(4):
                    nc.vector.tensor_tensor(out=c8v[lc // 2][:, lc % 2, :],
                                            in0=craw[lc][:], in1=rsr[:],
                                            op=mybir.AluOpType.mult)
                for lc in range(4):
                    ct = cts.tile([128, 512], BF16, tag=f"cT{lc}",
                                  name=f"cT{lc}")
                    nc.vector.tensor_tensor(out=ct[:], in0=craw[lc][:],
                                            in1=rsr[:],
                                            op=mybir.AluOpType.mult)
                    cT.append(ct)

            if sb == 0:
                qcb, qrotb, krotb = b0_proj()
                kabs_b = []
                for j in range(HPG):
                    pt = psg.tile([128, 512], FP32, tag="acc", name="kabb")
                    for lc in range(4):
                        nc.tensor.matmul(
                            pt[:],
                            wukbv[:, lc, j * 128:(j + 1) * 128],
                            cT[lc][:],
                            start=(lc == 0), stop=(lc == 3),
                        )
                    t = b0p.tile([128, 512], BF16, tag=f"kabb{j}",
                                 name=f"kabb{j}")
                    if j % 2 == 0:
                        nc.scalar.copy(t[:], pt[:])
                    else:
                        nc.vector.tensor_copy(t[:], pt[:])
                    kabs_b.append(t)
            for f in feats_b:
                qk_feature(f)

            # rope pack adds (Pool; q-planes skipped at sb=0)
            for t in rope_mul:
                t1, t2 = rope_mul[t]
                for hh in range(2):
                    j = (t % 2) * 2 + hh
                    pr = slice(hh * 64, (hh + 1) * 64)
                    if t < 2:
                        nc.gpsimd.tensor_tensor(
                            out=qpv[j][pr, qb, 1, :],
                            in0=t1[pr, :], in1=t2[pr, :],
                            op=mybir.AluOpType.add)
                    else:
                        nc.gpsimd.tensor_tensor(
                            out=kpv[j][pr, 4 * sb:4 * sb + 4, 1, :],
                            in0=t1[pr, :].rearrange("p (a b) -> p a b",
                                                    b=128),
                            in1=t2[pr, :].rearrange("p (a b) -> p a b",
                                                    b=128),
                            op=mybir.AluOpType.add)

            # prefetch next block's activations while s2/s3 run
            if sb + 1 < SB:
                xb_nxt = fetch_xb(sb + 1)
                x8_nxt = fetch_x8(sb + 1)
                tab_nxt = fetch_tabs(sb + 1)

            # ---- s2: absorption for this token block -----------------
            for j in range(HPG):
                pt = psg.tile([128, 512], FP32, tag="acc", name="kab")
                for q in range(2):
                    nc.tensor.matmul(
                        pt[:], wukv[:, q, j, :, :],
                        c8v[q][:, :, :],
                        start=(q == 0), stop=(q == 1), perf_mode=DR,
                    )
                nc.vector.tensor_copy(
                    kpv[j][:, 4 * sb:4 * sb + 4, 0, :],
                    pt[:].rearrange("p (a b) -> p a b", b=128))
            for tt in range(4):
                pt = psg.tile([128, 512], FP32, tag="acc", name="vab")
                for lc in range(4):
                    nc.tensor.matmul(
                        pt[:], cT[lc][:, tt * 128:(tt + 1) * 128],
                        wuvv[:, lc, :],
                        start=(lc == 0), stop=(lc == 3),
                    )
                nc.vector.tensor_copy(vabs[4 * sb + tt][:], pt[:])

            # ---- s3: attention for q-block sb ------------------------
            nti = 4 * sb + 4
            LAG = 2

            def s3_make(jpair):
                ja, jb = 2 * jpair, 2 * jpair + 1
                return {
                    "ja": ja, "jb": jb, "pfs": {}, "emitted": 0,
                    "cx": [psg.tile([128, 512], FP32, tag="acc",
                                    name=f"cx{j}") for j in (ja, jb)],
                    "sm": [psg.tile([128, 512], FP32, tag="acc",
                                    name=f"sm{j}") for j in (ja, jb)],
                }

            def s3_scores(st, ti):
                ja, jb = st["ja"], st["jb"]
                j0 = ti - 4 * sb
                c0 = max(j0, 0) * 128
                pfpair = []
                for j in (ja, jb):
                    scp = psg.tile([128, 512], FP32, tag="acc", name="sc")
                    if sb == 0:
                        tsl = slice(ti * 128, (ti + 1) * 128)
                        rsl = slice((j % 2) * 64, (j % 2) * 64 + 64)
                        nc.tensor.matmul(
                            scp[:, c0:512],
                            kabs_b[j][:, tsl], qcb[j][:, c0:512],
                            start=True, stop=False,
                        )
                        nc.tensor.matmul(
                            scp[:, c0:512],
                            krotb[j // 2][rsl, tsl],
                            qrotb[j // 2][rsl, c0:512],
                            start=False, stop=True,
                        )
                    else:
                        nc.tensor.matmul(
                            scp[:],
                            kpv[j][:, ti, :, :],
                            qpv[j][:, qb, :, :],
                            start=True, stop=True, perf_mode=DR,
                        )
                    pf = s3t.tile([128, 512], BF16, tag="pf", name="pf")
                    nc.scalar.activation(pf[:, c0:512], scp[:, c0:512],
                                         mybir.ActivationFunctionType.Exp,
                                         scale=SCALE)
                    if j0 >= 0:
                        nc.vector.tensor_tensor(
                            out=pf[:, c0:c0 + 128], in0=pf[:, c0:c0 + 128],
                            in1=mask_t[:], op=mybir.AluOpType.mult)
                    pfpair.append(pf)
                st["pfs"][ti] = (pfpair, c0)

            def s3_pv(st, tv):
                ja, jb = st["ja"], st["jb"]
                (pfa, pfb), c0 = st["pfs"].pop(tv)
                for (pf, cxp, smp, j) in ((pfa, st["cx"][0], st["sm"][0], ja),
                                          (pfb, st["cx"][1], st["sm"][1], jb)):
                    nc.tensor.matmul(
                        cxp[:, c0:512],
                        vabs[tv][:, j * 128:(j + 1) * 128],
                        pf[:, c0:512],
                        start=(tv == 0), stop=(tv == nti - 1),
                    )
                    nc.tensor.matmul(
                        smp[:, c0:512], ones_t[:], pf[:, c0:512],
                        start=(tv == 0), stop=(tv == nti - 1),
                    )

            def s3_run(st, tis):
                for ti in tis:
                    s3_scores(st, ti)
                    st["emitted"] += 1
                    if st["emitted"] > LAG:
                        s3_pv(st, ti - LAG)

            def s3_finish(st):
                for tv in sorted(st["pfs"].keys()):
                    s3_pv(st, tv)
                ja, jb = st["ja"], st["jb"]
                outs = []
                for i, j in enumerate((ja, jb)):
                    rec = s3r.tile([128, 512], FP32, tag="rec", name="rec")
                    nc.vector.reciprocal(rec[:], st["sm"][i][:])
                    ctx = ctxp.tile([128, 512], BF16, tag=f"ctx{j}",
                                    name=f"ctx{j}")
                    nc.vector.tensor_tensor(out=ctx[:], in0=st["cx"][i][:],
                                            in1=rec[:],
                                            op=mybir.AluOpType.mult)
                    outs.append(ctx)
                return outs

            stA = s3_make(0)
            s3_run(stA, range(nti))
            ctxA = s3_finish(stA)
            stB = s3_make(1)
            s3_run(stB, range(nti))
            ctxB = s3_finish(stB)
            ctx = ctxA + ctxB

            # ---- s4: output projection for this s block --------------
            for dt in range(16):
                dc = slice(dt * 128, (dt + 1) * 128)
                pt = psg.tile([128, 512], FP32, tag="acc", name="po")
                for j in range(HPG):
                    nc.tensor.matmul(pt[:], wo_t[j][:, dc], ctx[j][:],
                                     start=(j == 0), stop=(j == HPG - 1))
                orow = orp.tile([128, 512], BF16, tag="orow", name="orow")
                if dt % 2 == 0:
                    nc.scalar.copy(orow[:], pt[:])
                else:
                    nc.vector.tensor_copy(orow[:], pt[:])
                nc.sync.dma_start(outT[dc, sc], orow[:])

            if sb + 1 < SB:
                xb_cur, x8_cur, tab_cur = xb_nxt, x8_nxt, tab_nxt

        orp.release()
        ctxp.release()
        s3r.release()
        s3t.release()
        b0p.release()
        rsp.release()
        sqp.release()
        rope.release()
        c8s.release()
        crp.release()
        cts.release()
        psg.release()
        wqs.release()
        tabs.release()
        x8s.release()
        xs.release()
        vp.release()
        packs.release()
        wgt.release()
        cst.release()

    nc.compile()
    return nc


def _host_inputs(core, x, W_DKV, W_KRope, W_Q, W_UK, W_UV, W_O, kv_norm_w,
                 offset):
    b, g = core // HPG, core % HPG
    F8 = ml_dtypes.float8_e4m3
    BF = ml_dtypes.bfloat16

    xT = np.ascontiguousarray(x[b].T)  # [DIN, S] fp32
    x8 = np.ascontiguousarray(
        xT.reshape(8, 2, 128, S).transpose(2, 0, 1, 3)
    ).astype(F8).reshape(128, -1)

    qc_rows = W_Q[512 * g:512 * (g + 1)]
    qr_rows = W_Q[DOUT + 256 * g:DOUT + 256 * (g + 1)]
    kr_rows = W_KRope[256 * g:256 * (g + 1)]
    wqk = np.concatenate([qc_rows, qr_rows, kr_rows], axis=0)  # [1024, DIN]
    # [p, ftile, cp, i, f] with k = cp*256 + i*128 + p
    w8 = np.ascontiguousarray(
        wqk.T.reshape(8, 2, 128, 8, 128).transpose(2, 3, 0, 1, 4)
    ).astype(F8).reshape(128, -1)

    wukf = (W_UK[512 * g:512 * (g + 1)] * kv_norm_w[None, :]).T  # [L, 512]
    # [p, q, j, i, d] with l = q*256 + i*128 + p, col = j*128 + d
    wuk8 = np.ascontiguousarray(
        wukf.reshape(2, 2, 128, 4, 128).transpose(2, 0, 3, 1, 4)
    ).astype(F8).reshape(128, -1)
    wuvf = (W_UV[512 * g:512 * (g + 1)] * kv_norm_w[None, :]).T

    pos = (float(offset) + np.arange(S, dtype=np.float64))
    inv = 1.0 / ROPE_BASE ** (np.arange(0, RD, 2, dtype=np.float64) / RD)
    ang = pos[:, None] * inv[None, :]               # [S, 32]
    cos64 = np.concatenate([np.cos(ang), np.cos(ang)], axis=1).T  # [64,S]
    st64 = np.concatenate([-np.sin(ang), np.sin(ang)], axis=1).T
    ctab = np.concatenate([cos64, cos64], axis=0).astype(BF)
    stab = np.concatenate([st64, st64], axis=0).astype(BF)

    return {
        "xb": xT.astype(BF),
        "x8": x8,
        "wc": np.ascontiguousarray(W_DKV.T.astype(BF)),
        "w8": w8,
        "wqkb": np.ascontiguousarray(wqk.T.astype(BF)),
        "wuk8": wuk8,
        "wukb": np.ascontiguousarray(wukf.astype(BF)),
        "wuv": np.ascontiguousarray(wuvf.astype(BF)),
        "wo": np.ascontiguousarray(W_O[:, 512 * g:512 * (g + 1)].T.astype(BF)),
        "ctab": ctab,
        "stab": stab,
        "onesd": np.ones((128, 128), dtype=BF),
        "maskd": np.triu(np.ones((128, 128))).astype(BF),
        "epsd": np.full((128, 1), EPS, dtype=np.float32),
        "zed": np.zeros((128, S), dtype=F8),
    }


def _reference_numpy(x, W_DKV, W_KRope, W_Q, W_UK, W_UV, W_O, kv_norm_w,
                     offset):
    """Pure-numpy fallback (only used if offset != 0)."""
    x = x.astype(np.float64)
    b, s, _ = x.shape
    ckv = x @ W_DKV.T.astype(np.float64)
    ms = np.mean(ckv * ckv, axis=-1, keepdims=True)
    ckv = ckv / np.sqrt(ms + EPS) * kv_norm_w
    kr = (x @ W_KRope.T.astype(np.float64)).reshape(b, s, H, RD).transpose(0, 2, 1, 3)
    qt = x @ W_Q.T.astype(np.float64)
    qc = qt[..., :DOUT].reshape(b, s, H, HD).transpose(0, 2, 1, 3)
    qr = qt[..., DOUT:].reshape(b, s, H, RD).transpose(0, 2, 1, 3)

    def rope(t):
        pos = float(offset) + np.arange(s)
        inv = 1.0 / ROPE_BASE ** (np.arange(0, RD, 2) / RD)
        ang = pos[:, None] * inv[None, :]
        cos = np.concatenate([np.cos(ang)] * 2, -1)
        sin = np.concatenate([np.sin(ang)] * 2, -1)
        t1, t2 = t[..., :RD // 2], t[..., RD // 2:]
        rot = np.concatenate([-t2, t1], -1)
        return t * cos + rot * sin

    qr, kr = rope(qr), rope(kr)
    wuk = W_UK.reshape(H, HD, L).astype(np.float64)
    qa = np.einsum("bhsd,hdl->bhsl", qc, wuk)
    sc = (np.einsum("bhsl,btl->bhst", qa, ckv)
          + np.einsum("bhsr,bhtr->bhst", qr, kr)) / math.sqrt(HD + RD)
    causal = np.arange(s)[None, :] > (np.arange(s)[:, None] + int(offset))
    sc = np.where(causal[None, None], -np.inf, sc)
    sc = sc - sc.max(-1, keepdims=True)
    e = np.exp(sc)
    attn = e / e.sum(-1, keepdims=True)
    cl = np.einsum("bhst,btl->bhsl", attn, ckv)
    wuv = W_UV.reshape(H, HD, L).astype(np.float64)
    ctx = np.einsum("bhsl,hdl->bhsd", cl, wuv)
    ctx = ctx.transpose(0, 2, 1, 3).reshape(b, s, DOUT)
    return (ctx @ W_O.T.astype(np.float64)).astype(np.float32)


def kernel(**inputs) -> np.ndarray:
    args = {k: np.asarray(v) if not np.isscalar(v) else v
            for k, v in inputs.items()}
    x = np.asarray(args["x"], dtype=np.float32)
    offset = int(np.asarray(args["offset"]))
    names = ["x", "W_DKV", "W_KRope", "W_Q", "W_UK", "W_UV", "W_O",
             "kv_norm_w"]
    arrs = {n: np.asarray(args[n], dtype=np.float32) for n in names}

    if offset != 0:
        return _reference_numpy(**arrs, offset=offset)

    if "nc" not in _cache:
        _cache["nc"] = _build()
    nc = _cache["nc"]

    from concourse.bass_utils import run_bass_kernel_spmd

    in_maps = [
        _host_inputs(core, offset=offset, **arrs) for core in range(NCORES)
    ]
    res = run_bass_kernel_spmd(nc, in_maps, list(range(NCORES)))
    _cache["last_result"] = res

    out = np.zeros((B, S, DIN), dtype=np.float32)
    for core in range(NCORES):
        b = core // HPG
        out[b] += res.results[core]["outT"].T.astype(np.float32)
    return out
